# revision 21
# baseline (speedup 1.0000x reference)
"""BCQLinear (3-bit binary-coded quantized linear) Trainium2 kernel.

Full-input contract: kernel(**inputs) takes the unsharded inputs of
nn_BCQLinear_88510686036218 and returns the full [1, 128, 4096] output.

Math: w = alpha*(2*S-7) + beta with S in [0,8) the 3-bit code, then
y = (x[:, in_reorder] @ w)[:, out_reorder].
Rewritten: w = A2*S + B with A2 = 2*alpha, B = beta - 7*alpha.

Sharding: out-features split 8 ways (512 cols/core), x replicated.

Per-core device program (SPMD, one Bass program):
  - Contraction rows are band-packed: K-tile kt in [0,32), partition p:
    dequant row i(kt,p) = 128*(p//4) + 4*kt + (p%4), so a single [128,512]
    alpha tile (A2_rep[p,:] = A2[p//4,:]) serves every K-tile.
  - Codes arrive as packed int32 words holding 4 PAIRS of 3-bit fields:
    pair r at bits [3r,3r+3) (lo) and [16+3r,16+3r+3) (hi).  One two-op
    tensor_scalar (>>3r, &0x70007) extracts a pair per element; the
    result buffer reinterpreted as int16 is a dense stream of codes, so
    the dequant multiply (codes x fp16 alpha) runs in the DVE 16-bit
    2x perf mode, and the matmuls run fp16 (full PE rate, fp32 PSUM).
  - The beta part is y += xsum @ B with xsum[g,t] per-group sums of
    permuted x (host-computed; 0.01% of the FLOPs) via one fp32 K=32
    matmul issued FIRST in the PSUM accumulation group.
  - Inputs stream as small contiguous DRAM tensors spread over four DMA
    queues (sync/scalar/tensor/gpsimd); a warm-up matmul burst runs
    during the DMA window to release the PE HAM clock-gate; the output
    leaves as two pipelined fp16 halves.
"""
import numpy as np
from contextlib import ExitStack

import concourse.bass as bass
import concourse.mybir as mybir
import concourse.tile as tile
from concourse import bacc

IN_F, OUT_F, WBITS, GS, OFI = 4096, 4096, 3, 128, 128
NG, NB = 32, 32
NCORES = 8
OPC = OUT_F // NCORES        # 512 out-cols per core
NKT = 32                     # K-tiles of 128 rows
NR = 4                       # field-pairs per int32 word
NWC = OPC // (2 * NR)        # 64 packed words per (partition, K-tile)
T = 128                      # tokens
NCHUNK = 4                   # w/unpack pipeline chunks
KTC = NKT // NCHUNK          # K-tiles per chunk
NBAT = 2                     # K-tiles per dequant-multiply batch
NWARM = 10                   # PE warm-up matmuls

F32 = mybir.dt.float32
F16 = mybir.dt.float16
I32 = mybir.dt.int32
I16 = mybir.dt.int16
ALU = mybir.AluOpType

_PROGRAM_CACHE = {}


# ---------------------------------------------------------------- host prep
def _dequant_codes(qweight):
    """S[i, o] in [0,8): w = alpha*(2S-7)+beta."""
    qw = np.asarray(qweight, dtype=np.uint32).reshape(NG, NB, WBITS, GS * OFI // 32)
    bits = (qw[..., None] >> np.arange(32, dtype=np.uint32)) & 1
    bits = bits.reshape(NG, NB, WBITS, GS, OFI)
    S = (bits * (2 ** np.arange(WBITS, dtype=np.uint32))[:, None, None]).sum(axis=2)
    return S.transpose(0, 2, 1, 3).reshape(IN_F, OUT_F).astype(np.uint32)


def _band_rows():
    kt, p = np.meshgrid(np.arange(NKT), np.arange(128), indexing="ij")
    return 128 * (p // 4) + 4 * kt + (p % 4)      # [NKT, 128]


def _prepare(inputs):
    x = np.asarray(inputs["x"], np.float32).reshape(-1, IN_F)
    alpha = np.asarray(inputs["alpha"], np.float32)
    beta = np.asarray(inputs["beta"], np.float32)
    in_reorder = np.asarray(inputs["in_reorder"], np.int64)
    xf = x[:, in_reorder]

    S = _dequant_codes(inputs["qweight"])          # [IN_F, OUT_F] uint32
    A2full = (2.0 * alpha).astype(np.float16)
    Bfull = (beta.astype(np.float64) - 7.0 * alpha.astype(np.float64)
             ).astype(np.float32)

    rows = _band_rows()                            # [NKT, 128]
    XT = np.ascontiguousarray(
        xf[:, rows.reshape(-1)].T.reshape(NKT, 128, T).transpose(1, 0, 2)
    ).reshape(128, NKT * T).astype(np.float16)     # [p, kt*T]
    # per-group token sums (beta part): xsumT[g, t]
    xsumT = (xf.reshape(T, NG, GS).sum(axis=2, dtype=np.float64)
             .T.astype(np.float32))

    CW = KTC * NWC                                 # 512 words/chunk/partition
    XQ = NKT * T // NCHUNK
    in_maps = []
    for c in range(NCORES):
        cols = slice(OPC * c, OPC * (c + 1))
        # codes for this core in banded row order: [p, kt, o']
        Sc = S[rows.reshape(-1), cols].reshape(NKT, 128, OPC).transpose(1, 0, 2)
        # pack pairs: o' = r*128 + 2c' + h  ->  bits [3r+16h, +3)
        W = np.zeros((128, NKT, NWC), np.uint32)
        for r in range(NR):
            for h in range(2):
                W |= Sc[:, :, r * 128 + h::2][:, :, :NWC] << (3 * r + 16 * h)
        W = W.reshape(128, NKT * NWC)
        a2rep = A2full[np.arange(128) // 4][:, cols]
        consts32 = np.zeros((NG, OPC + T), np.float32)
        consts32[:, :OPC] = Bfull[:, cols]
        consts32[:, OPC:] = xsumT
        a2d = np.tile(a2rep, (1, NBAT))
        im = dict(consts32=consts32)
        for h in range(2):
            im[f"a2_{h}"] = np.ascontiguousarray(a2d[:, h * OPC:(h + 1) * OPC])
        for ch in range(NCHUNK):
            wc = W[:, ch * CW:(ch + 1) * CW]
            for q in range(4):
                im[f"w{ch}_{q}"] = np.ascontiguousarray(
                    wc[:, q * (CW // 4):(q + 1) * (CW // 4)]).view(np.int32)
            xc = XT[:, ch * XQ:(ch + 1) * XQ]
            for h in range(2):
                im[f"xt{ch}_{h}"] = np.ascontiguousarray(
                    xc[:, h * (XQ // 2):(h + 1) * (XQ // 2)])
        in_maps.append(im)
    return in_maps


# ---------------------------------------------------------------- program
def build_program():
    nc = bacc.Bacc("TRN2")
    CW = KTC * NWC            # packed words per chunk per partition
    CQ = CW // 4              # words per w sub-DMA
    XQ = NKT * T // NCHUNK    # xt columns per chunk

    w_dr = {(ch, q): nc.declare_dram_parameter(f"w{ch}_{q}", [128, CQ], I32,
                                               isOutput=False)
            for ch in range(NCHUNK) for q in range(4)}
    xt_dr = {(ch, h): nc.declare_dram_parameter(f"xt{ch}_{h}", [128, XQ // 2],
                                                F16, isOutput=False)
             for ch in range(NCHUNK) for h in range(2)}
    a2_dr = [nc.declare_dram_parameter(f"a2_{h}", [128, OPC], F16,
                                       isOutput=False) for h in range(2)]
    consts32 = nc.declare_dram_parameter("consts32", [NG, OPC + T], F32,
                                         isOutput=False)
    z = nc.declare_dram_parameter("z", [T, OPC], F16, isOutput=True)

    with tile.TileContext(nc) as tc, ExitStack() as ctx:
        cpool = ctx.enter_context(tc.tile_pool(name="const", bufs=1))
        wmpool = ctx.enter_context(tc.tile_pool(name="wm", bufs=3))
        opool = ctx.enter_context(tc.tile_pool(name="out", bufs=1))
        ppool = ctx.enter_context(tc.tile_pool(name="psum", bufs=1, space="PSUM"))

        # --- input tiles -------------------------------------------------
        w_sb = [cpool.tile([128, CW], I32, tag=f"w{ch}", name=f"wsb{ch}")
                for ch in range(NCHUNK)]
        xt_sb = [cpool.tile([128, XQ], F16, tag=f"xt{ch}", name=f"xtsb{ch}")
                 for ch in range(NCHUNK)]
        a2_sb = cpool.tile([128, NBAT * OPC], F16, tag="a2")
        c32_sb = cpool.tile([NG, OPC + T], F32, tag="c32")
        bm_sb = c32_sb[:, :OPC]
        xs_sb = c32_sb[:, OPC:]

        # --- DMA schedule: need-ordered round-robin over 3 queues --------
        XH = XQ // 2
        pieces = []
        for q in range(4):
            pieces.append((w_sb[0][:, q * CQ:(q + 1) * CQ], w_dr[0, q][:]))
        for h in range(2):
            pieces.append((a2_sb[:, h * OPC:(h + 1) * OPC], a2_dr[h][:]))
        for h in range(2):
            pieces.append((xt_sb[0][:, h * XH:(h + 1) * XH], xt_dr[0, h][:]))
        for ch in range(1, NCHUNK):
            for q in range(4):
                pieces.append((w_sb[ch][:, q * CQ:(q + 1) * CQ], w_dr[ch, q][:]))
            for h in range(2):
                pieces.append((xt_sb[ch][:, h * XH:(h + 1) * XH],
                               xt_dr[ch, h][:]))
        pieces.append((c32_sb[:], consts32[:]))
        queues = [nc.sync, nc.scalar, nc.gpsimd]
        for i, (dst, src) in enumerate(pieces):
            queues[i % 3].dma_start(out=dst, in_=src)

        # --- PE warm-up during the DMA window ----------------------------
        wu_sb = cpool.tile([128, OPC], F16, tag="wu")
        nc.gpsimd.memset(wu_sb[:], 0.0)
        psum_wu = ppool.tile([128, OPC], F32, tag="wu_ps")
        for i in range(NWARM):
            nc.tensor.matmul(psum_wu[:], wu_sb[:, :T], wu_sb[:],
                             start=True, stop=True)

        v_sb = [cpool.tile([128, NR * CW], I32, tag=f"v{ch}", name=f"v{ch}")
                for ch in range(NCHUNK)]

        # --- main pipeline ----------------------------------------------
        psum_main = ppool.tile([T, OPC], F32, tag="main")
        # beta part first: psum = xsumT.T @ B  (fp32, K=32)
        nc.tensor.matmul(psum_main[:], xs_sb, bm_sb, start=True, stop=False)
        for ch in range(NCHUNK):
            # unpack pair r: V32[p, r*CW + k*NWC + c] = (W >> 3r) & 0x70007
            for r in range(NR):
                nc.vector.tensor_scalar(
                    v_sb[ch][:, r * CW:(r + 1) * CW],
                    w_sb[ch][:],
                    3 * r,
                    0x00070007,
                    ALU.logical_shift_right,
                    ALU.bitwise_and,
                )
            # int16 view: [p, (r, k, q)], q = 2c+h in [0,256), o' = r*128+q
            v16 = v_sb[ch][:].bitcast(I16).rearrange(
                "p (r k q) -> p k r q", r=NR, k=KTC, q=2 * NWC)
            for b in range(KTC // NBAT):
                # dequant-multiply NBAT K-tiles in one 16-bit 2x-mode op
                wm = wmpool.tile([128, NBAT * OPC], F16, tag="wm")
                nc.vector.tensor_tensor(
                    wm[:].rearrange("p (k r q) -> p k r q", k=NBAT, r=NR),
                    v16[:, b * NBAT:(b + 1) * NBAT],
                    a2_sb[:].rearrange("p (k r q) -> p k r q", k=NBAT, r=NR),
                    ALU.mult,
                )
                for j in range(NBAT):
                    kt = ch * KTC + b * NBAT + j
                    nc.tensor.matmul(
                        psum_main[:],
                        xt_sb[ch][:, (b * NBAT + j) * T:(b * NBAT + j + 1) * T],
                        wm[:, j * OPC:(j + 1) * OPC],
                        start=False,
                        stop=(kt == NKT - 1),
                    )
        # --- output: two pipelined fp16 halves ---------------------------
        out_sb = opool.tile([T, OPC], F16, tag="out_sb")
        HALF = OPC // 2
        nc.scalar.copy(out=out_sb[:, :HALF], in_=psum_main[:, :HALF])
        nc.sync.dma_start(out=z[:, :HALF], in_=out_sb[:, :HALF])
        nc.scalar.copy(out=out_sb[:, HALF:], in_=psum_main[:, HALF:])
        nc.gpsimd.dma_start(out=z[:, HALF:], in_=out_sb[:, HALF:])
    nc.finalize()
    return nc


def _get_program():
    if "nc" not in _PROGRAM_CACHE:
        _PROGRAM_CACHE["nc"] = build_program()
    return _PROGRAM_CACHE["nc"]


# ---------------------------------------------------------------- entry
def kernel(**inputs):
    from concourse.bass_utils import run_bass_kernel_spmd

    in_maps = _prepare(inputs)
    nc = _get_program()
    res = run_bass_kernel_spmd(nc, in_maps, list(range(NCORES)))
    z = np.concatenate(
        [res.results[c]["z"].astype(np.float32) for c in range(NCORES)], axis=1)
    out_reorder = np.asarray(inputs["out_reorder"], np.int64)
    y = z[:, out_reorder].reshape(1, T, OUT_F).astype(np.float32)
    return y
